# revision 6
# baseline (speedup 1.0000x reference)
"""Trainium2 Bass kernel for a margin-softmax cross-entropy loss.

Reference computation (B=4096, D=512, C=10575):
    original = feats @ w                         # [B, C]
    means    = centers / counts[:, None]
    mn       = means / ||means||                 # unit rows
    dists    = mn[labels] @ mn.T                 # [B, C]
    logits   = original + onehot(labels) * dists # only label column changes
    loss     = mean(CE(logits, labels))

Key identities used:
  * Only the label column of `dists` survives the onehot mask, and
    dists[i, labels[i]] = mn[labels[i]] . mn[labels[i]] = |mn|^2 ~ 1.0
    (computed per-class on host from centers/counts; the BxC `dists`
    GEMM never needs to run).
  * logits are bounded (|logit| < ~3), so sum(exp(logits)) needs no
    max-subtraction; CE = log(sum_j exp(l_j)) - l_label.
  * Cross-entropy needs only two per-row scalars from the big [B, C]
    logits matrix: S_i = sum_j exp(l_ij) and t_i = l_i,label(i).

Device work per core (classes sharded 8 ways, ~1322 classes/core padded
to 1536): one [4096 x 512] @ [512 x 1536] bf16 GEMM with exp+row-sum
fused on the Scalar engine (activation accum_out), plus a tiny
"diagonal GEMM" for the label logits: with wlab = w[:, labels] gathered
on host for this core's 512-row batch slice, diag(featsT.T @ wlab) is
exactly t_i; the diagonal is pulled out with a constant identity mask
(one multiply + one reduce on 128x128 tiles).

Host combines the 8 partial sums (trivial [4096]-vector math) and
applies the margin correction:
    S' = S - exp(t) + exp(t + d);  nll = log(S') - (t + d).
Zero-padded w columns produce logits == 0 exactly, contributing
exp(0) = 1 each; the host subtracts the pad count from S.
"""

from contextlib import ExitStack

import ml_dtypes
import numpy as np

import concourse.bass as bass
import concourse.tile as tile
from concourse import bacc, mybir
from concourse.bass_utils import run_bass_kernel_spmd

B = 4096
D = 512
C = 10575
NCORES = 8
CS_BASE = 1322        # real classes on cores 0..6; core 7 gets 1321
CSH = 1328            # padded per-core class count
CW = (512, 512, 304)  # class-tile widths (PSUM bank-aligned starts)
CO = (0, 512, 1024)   # class-tile offsets
KT = D // 128         # 4 contraction tiles
BT = B // 128         # 32 batch tiles
BSH = B // NCORES     # 512 rows of label logits per core
JT = BSH // 128       # 4 diagonal sub-tiles
FCH = 8               # fT DMA chunks per k-tile

BF16 = mybir.dt.bfloat16
F32 = mybir.dt.float32

_CACHE = {}


def _build_nc():
    nc = bacc.Bacc("TRN2", debug=False, target_bir_lowering=False)

    fT = nc.dram_tensor("fT", [KT, 128, B], BF16, kind="ExternalInput").ap()
    wS = nc.dram_tensor("wS", [KT, 128, CSH], BF16, kind="ExternalInput").ap()
    wL = nc.dram_tensor("wL", [KT, 128, BSH], BF16, kind="ExternalInput").ap()
    fS = nc.dram_tensor("fS", [KT, 128, BSH], BF16, kind="ExternalInput").ap()
    ident = nc.dram_tensor("ident", [128, 128], F32, kind="ExternalInput").ap()
    outS = nc.dram_tensor("outS", [BT, 128, 1], F32, kind="ExternalOutput").ap()
    outT = nc.dram_tensor("outT", [128, JT], F32, kind="ExternalOutput").ap()

    with tile.TileContext(nc) as tc, ExitStack() as ctx:
        consts = ctx.enter_context(tc.tile_pool(name="consts", bufs=1))
        psums = ctx.enter_context(tc.tile_pool(name="psums", bufs=2, space="PSUM"))
        psumd = ctx.enter_context(tc.tile_pool(name="psumd", bufs=2, space="PSUM"))
        work = ctx.enter_context(tc.tile_pool(name="work", bufs=3))
        outs = ctx.enter_context(tc.tile_pool(name="outs", bufs=3))

        # weights + diag-GEMM inputs first: PE's first matmuls need them
        wS_sb = []
        for k in range(KT):
            t = consts.tile([128, CSH], BF16, tag=f"wS{k}")
            nc.sync.dma_start(out=t[:], in_=wS[k])
            wS_sb.append(t)
        wL_sb = []
        for k in range(KT):
            t = consts.tile([128, BSH], BF16, tag=f"wL{k}")
            nc.sync.dma_start(out=t[:], in_=wL[k])
            wL_sb.append(t)
        fS_sb = []
        for k in range(KT):
            t = consts.tile([128, BSH], BF16, tag=f"fS{k}")
            nc.sync.dma_start(out=t[:], in_=fS[k])
            fS_sb.append(t)
        id_sb = consts.tile([128, 128], F32, tag="ident")
        nc.sync.dma_start(out=id_sb[:], in_=ident[:])
        # feats transposed, chunked so early b-tiles unblock quickly
        fT_sb = []
        bch = B // FCH
        for k in range(KT):
            t = consts.tile([128, B], BF16, tag=f"fT{k}")
            fT_sb.append(t)
        for j in range(FCH):
            for k in range(KT):
                nc.sync.dma_start(
                    out=fT_sb[k][:, j * bch:(j + 1) * bch],
                    in_=fT[k][:, j * bch:(j + 1) * bch],
                )

        # label logits: diag(fS.T @ wL) on 128x128 sub-tiles
        tt = outs.tile([128, JT], F32, tag="tt")
        for j in range(JT):
            pd = psumd.tile([128, 128], F32, tag="pd")
            for k in range(KT):
                nc.tensor.matmul(
                    out=pd[:],
                    lhsT=fS_sb[k][:, j * 128:(j + 1) * 128],
                    rhs=wL_sb[k][:, j * 128:(j + 1) * 128],
                    start=(k == 0),
                    stop=(k == KT - 1),
                )
            scr = work.tile([128, 128], F32, tag="scr")
            nc.vector.tensor_mul(out=scr[:], in0=id_sb[:], in1=pd[:])
            nc.vector.tensor_reduce(
                out=tt[:, j:j + 1], in_=scr[:],
                axis=mybir.AxisListType.X, op=mybir.AluOpType.add,
            )
        nc.sync.dma_start(out=outT[:], in_=tt[:])

        # main GEMM + exp row-sums (one wide ACT per b-tile strip)
        for b in range(BT):
            ps = psums.tile([128, CSH], F32, tag="ps")
            st = outs.tile([128, 1], F32, tag="st")
            for k in range(KT):
                for c in range(len(CW)):
                    nc.tensor.matmul(
                        out=ps[:, CO[c]:CO[c] + CW[c]],
                        lhsT=fT_sb[k][:, b * 128:(b + 1) * 128],
                        rhs=wS_sb[k][:, CO[c]:CO[c] + CW[c]],
                        start=(k == 0),
                        stop=(k == KT - 1),
                    )
            e = work.tile([128, CSH], BF16, tag="e")
            nc.scalar.activation(
                out=e[:],
                in_=ps[:],
                func=mybir.ActivationFunctionType.Exp,
                accum_out=st[:],
            )
            nc.sync.dma_start(out=outS[b], in_=st[:])

    nc.compile()
    return nc


def _core_sizes():
    sizes = [CS_BASE] * (NCORES - 1) + [C - CS_BASE * (NCORES - 1)]
    starts = np.concatenate([[0], np.cumsum(sizes)[:-1]]).astype(np.int64)
    return np.array(sizes, dtype=np.int64), starts


def _prepare_inputs(feats, labels, w):
    sizes, starts = _core_sizes()
    fT_host = np.ascontiguousarray(
        feats.reshape(B, KT, 128).transpose(1, 2, 0)
    ).astype(ml_dtypes.bfloat16)
    ident = np.eye(128, dtype=np.float32)

    in_maps = []
    for p in range(NCORES):
        c0, sz = int(starts[p]), int(sizes[p])
        wp = np.zeros((D, CSH), dtype=np.float32)
        wp[:, :sz] = w[:, c0:c0 + sz]
        wS_host = np.ascontiguousarray(
            wp.reshape(KT, 128, CSH)
        ).astype(ml_dtypes.bfloat16)

        rows = slice(p * BSH, (p + 1) * BSH)
        wlab = w[:, labels[rows]]                     # [D, BSH]
        wL_host = np.ascontiguousarray(
            wlab.reshape(KT, 128, BSH)
        ).astype(ml_dtypes.bfloat16)
        fsel = feats[rows]                            # [BSH, D]
        fS_host = np.ascontiguousarray(
            fsel.reshape(BSH, KT, 128).transpose(1, 2, 0)
        ).astype(ml_dtypes.bfloat16)

        in_maps.append({
            "fT": fT_host,
            "wS": wS_host,
            "wL": wL_host,
            "fS": fS_host,
            "ident": ident,
        })
    return in_maps


def _run(in_maps, trace=False):
    if "nc" not in _CACHE:
        _CACHE["nc"] = _build_nc()
    nc = _CACHE["nc"]
    return run_bass_kernel_spmd(
        nc, in_maps, core_ids=list(range(NCORES)), trace=trace
    )


def kernel(feats, labels, centers, counts, w, _trace=False, _ret_res=False):
    feats = np.asarray(feats, dtype=np.float32)
    labels_i = np.asarray(labels).astype(np.int64)
    centers = np.asarray(centers, dtype=np.float32)
    counts = np.asarray(counts, dtype=np.float32)
    w = np.asarray(w, dtype=np.float32)

    in_maps = _prepare_inputs(feats, labels_i, w)
    res = _run(in_maps, trace=_trace)

    sizes, starts = _core_sizes()

    # margin d_c = |means_c / ||means_c|| |^2 (~1.0), matching the reference's
    # f32 normalize-then-dot on the label diagonal
    means = (centers / counts[:, None]).astype(np.float32)
    nrm = np.sqrt((means.astype(np.float32) ** 2).sum(axis=1, keepdims=True))
    mn = (means / nrm).astype(np.float32)
    dsq = (mn.astype(np.float64) ** 2).sum(axis=1)       # [C]
    d = dsq[labels_i]                                    # [B]

    S_tot = np.zeros(B, dtype=np.float64)
    t = np.empty(B, dtype=np.float64)
    for p in range(NCORES):
        S_p = res.results[p]["outS"].astype(np.float64).reshape(B)
        S_tot += S_p - float(CSH - sizes[p])             # remove zero-pad exp(0)=1
        # outT[q, j] is row p*BSH + j*128 + q
        T_p = res.results[p]["outT"].astype(np.float64)  # [128, JT]
        t[p * BSH:(p + 1) * BSH] = T_p.T.reshape(BSH)

    z = S_tot - np.exp(t) + np.exp(t + d)
    nll = np.log(z) - (t + d)
    loss = np.float32(nll.mean())
    out = np.array(loss, dtype=np.float32)
    if _ret_res:
        return out, res
    return out


# revision 11
# speedup vs baseline: 1.5065x; 1.5065x over previous
"""Trainium2 Bass kernel for a margin-softmax cross-entropy loss.

Reference computation (B=4096, D=512, C=10575):
    original = feats @ w                         # [B, C]
    means    = centers / counts[:, None]
    mn       = means / ||means||                 # unit rows
    dists    = mn[labels] @ mn.T                 # [B, C]
    logits   = original + onehot(labels) * dists # only label column changes
    loss     = mean(CE(logits, labels))

Key identities used:
  * Only the label column of `dists` survives the onehot mask, and
    dists[i, labels[i]] = mn[labels[i]] . mn[labels[i]] = |mn|^2 ~ 1.0
    (computed per-class on host from centers/counts; the BxC `dists`
    GEMM never needs to run).
  * logits are bounded (|logit| < ~3), so sum(exp(logits)) needs no
    max-subtraction; CE = log(sum_j exp(l_j)) - l_label.
  * Cross-entropy needs only two per-row scalars from the big [B, C]
    logits matrix: S_i = sum_j exp(l_ij) and t_i = l_i,label(i).

Device work per core (classes sharded 8 ways, ~1322 classes/core padded
to 1536): one [4096 x 512] @ [512 x 1536] bf16 GEMM with exp+row-sum
fused on the Scalar engine (activation accum_out), plus a tiny
"diagonal GEMM" for the label logits: with wlab = w[:, labels] gathered
on host for this core's 512-row batch slice, diag(featsT.T @ wlab) is
exactly t_i; the diagonal is pulled out with a constant identity mask
(one multiply + one reduce on 128x128 tiles).

Host combines the 8 partial sums (trivial [4096]-vector math) and
applies the margin correction:
    S' = S - exp(t) + exp(t + d);  nll = log(S') - (t + d).
Zero-padded w columns produce logits == 0 exactly, contributing
exp(0) = 1 each; the host subtracts the pad count from S.
"""

from contextlib import ExitStack

import ml_dtypes
import numpy as np

import concourse.bass as bass
import concourse.tile as tile
from concourse import bacc, mybir
from concourse.bass_utils import run_bass_kernel_spmd

B = 4096
D = 512
C = 10575
NCORES = 8
CS_BASE = 1322        # real classes on cores 0..6; core 7 gets 1321
CSH = 1328            # padded per-core class count
CW = (512, 512, 304)  # class-tile widths (PSUM bank-aligned starts)
CO = (0, 512, 1024)   # class-tile offsets
KT = D // 128         # 4 contraction tiles
BT = B // 128         # 32 batch tiles
BSH = B // NCORES     # 512 rows of label logits per core
JT = BSH // 128       # 4 diagonal sub-tiles
FCH = 2               # fT DMA chunks per k-tile

BF16 = mybir.dt.bfloat16
F32 = mybir.dt.float32

_CACHE = {}


def _build_nc():
    nc = bacc.Bacc("TRN2", debug=False, target_bir_lowering=False)

    fT = nc.dram_tensor("fT", [KT, 128, B], BF16, kind="ExternalInput").ap()
    wS = nc.dram_tensor("wS", [KT, 128, CSH], BF16, kind="ExternalInput").ap()
    # wL and fS concatenated: [KT, 128, 0:BSH]=wL, [KT, 128, BSH:2*BSH]=fS
    wLfS = nc.dram_tensor("wLfS", [KT, 128, 2 * BSH], BF16, kind="ExternalInput").ap()
    ident = nc.dram_tensor("ident", [128, 128], F32, kind="ExternalInput").ap()
    outS = nc.dram_tensor("outS", [128, BT], F32, kind="ExternalOutput").ap()
    outT = nc.dram_tensor("outT", [128, JT], F32, kind="ExternalOutput").ap()

    with tile.TileContext(nc) as tc, ExitStack() as ctx:
        consts = ctx.enter_context(tc.tile_pool(name="consts", bufs=1))
        psums = ctx.enter_context(tc.tile_pool(name="psums", bufs=2, space="PSUM"))
        psumd = ctx.enter_context(tc.tile_pool(name="psumd", bufs=2, space="PSUM"))
        work = ctx.enter_context(tc.tile_pool(name="work", bufs=3))
        outs = ctx.enter_context(tc.tile_pool(name="outs", bufs=1))

        # scalar-engine HWDGE ring: weights + diag inputs (small, needed first)
        wS_sb = []
        for k in range(KT):
            t = consts.tile([128, CSH], BF16, tag=f"wS{k}")
            nc.scalar.dma_start(out=t[:], in_=wS[k])
            wS_sb.append(t)
        wLfS_sb = []
        for k in range(KT):
            t = consts.tile([128, 2 * BSH], BF16, tag=f"wLfS{k}")
            nc.scalar.dma_start(out=t[:], in_=wLfS[k])
            wLfS_sb.append(t)
        id_sb = consts.tile([128, 128], F32, tag="ident")
        nc.scalar.dma_start(out=id_sb[:], in_=ident[:])
        wL_sb = [t[:, 0:BSH] for t in wLfS_sb]
        fS_sb = [t[:, BSH:2 * BSH] for t in wLfS_sb]
        # sync-engine HWDGE ring: feats transposed, two halves per k-tile so
        # the first b-tiles unblock at ~half the load time
        fT_sb = []
        bch = B // FCH
        for k in range(KT):
            t = consts.tile([128, B], BF16, tag=f"fT{k}")
            fT_sb.append(t)
        for j in range(FCH):
            for k in range(KT):
                nc.sync.dma_start(
                    out=fT_sb[k][:, j * bch:(j + 1) * bch],
                    in_=fT[k][:, j * bch:(j + 1) * bch],
                )

        # label logits: diag(fS.T @ wL) on 128x128 sub-tiles
        tt = outs.tile([128, JT], F32, tag="tt")
        for j in range(JT):
            pd = psumd.tile([128, 128], F32, tag="pd")
            for k in range(KT):
                nc.tensor.matmul(
                    out=pd[:],
                    lhsT=fS_sb[k][:, j * 128:(j + 1) * 128],
                    rhs=wL_sb[k][:, j * 128:(j + 1) * 128],
                    start=(k == 0),
                    stop=(k == KT - 1),
                )
            scr = work.tile([128, 128], F32, tag="scr")
            nc.vector.tensor_mul(out=scr[:], in0=id_sb[:], in1=pd[:])
            nc.vector.tensor_reduce(
                out=tt[:, j:j + 1], in_=scr[:],
                axis=mybir.AxisListType.X, op=mybir.AluOpType.add,
            )
        nc.sync.dma_start(out=outT[:], in_=tt[:])

        # main GEMM + exp row-sums (one wide ACT per b-tile strip);
        # per-b sums land in one [128, BT] tile, stored with a single DMA
        st = outs.tile([128, BT], F32, tag="st")
        for b in range(BT):
            ps = psums.tile([128, CSH], F32, tag="ps")
            for k in range(KT):
                for c in range(len(CW)):
                    nc.tensor.matmul(
                        out=ps[:, CO[c]:CO[c] + CW[c]],
                        lhsT=fT_sb[k][:, b * 128:(b + 1) * 128],
                        rhs=wS_sb[k][:, CO[c]:CO[c] + CW[c]],
                        start=(k == 0),
                        stop=(k == KT - 1),
                    )
            e = work.tile([128, CSH], BF16, tag="e")
            nc.scalar.activation(
                out=e[:],
                in_=ps[:],
                func=mybir.ActivationFunctionType.Exp,
                accum_out=st[:, b:b + 1],
            )
        nc.sync.dma_start(out=outS[:], in_=st[:])

    nc.compile()
    return nc


def _core_sizes():
    sizes = [CS_BASE] * (NCORES - 1) + [C - CS_BASE * (NCORES - 1)]
    starts = np.concatenate([[0], np.cumsum(sizes)[:-1]]).astype(np.int64)
    return np.array(sizes, dtype=np.int64), starts


def _prepare_inputs(feats, labels, w):
    sizes, starts = _core_sizes()
    fT_host = np.ascontiguousarray(
        feats.reshape(B, KT, 128).transpose(1, 2, 0)
    ).astype(ml_dtypes.bfloat16)
    ident = np.eye(128, dtype=np.float32)

    in_maps = []
    for p in range(NCORES):
        c0, sz = int(starts[p]), int(sizes[p])
        wp = np.zeros((D, CSH), dtype=np.float32)
        wp[:, :sz] = w[:, c0:c0 + sz]
        wS_host = np.ascontiguousarray(
            wp.reshape(KT, 128, CSH)
        ).astype(ml_dtypes.bfloat16)

        rows = slice(p * BSH, (p + 1) * BSH)
        wlab = w[:, labels[rows]].reshape(KT, 128, BSH)           # [D, BSH]
        fsel = feats[rows].reshape(BSH, KT, 128).transpose(1, 2, 0)
        wLfS_host = np.concatenate([wlab, fsel], axis=2)
        wLfS_host = np.ascontiguousarray(wLfS_host).astype(ml_dtypes.bfloat16)

        in_maps.append({
            "fT": fT_host,
            "wS": wS_host,
            "wLfS": wLfS_host,
            "ident": ident,
        })
    return in_maps


def _run(in_maps, trace=False):
    if "nc" not in _CACHE:
        _CACHE["nc"] = _build_nc()
    nc = _CACHE["nc"]
    return run_bass_kernel_spmd(
        nc, in_maps, core_ids=list(range(NCORES)), trace=trace
    )


def kernel(feats, labels, centers, counts, w, _trace=False, _ret_res=False):
    feats = np.asarray(feats, dtype=np.float32)
    labels_i = np.asarray(labels).astype(np.int64)
    centers = np.asarray(centers, dtype=np.float32)
    counts = np.asarray(counts, dtype=np.float32)
    w = np.asarray(w, dtype=np.float32)

    in_maps = _prepare_inputs(feats, labels_i, w)
    res = _run(in_maps, trace=_trace)

    sizes, starts = _core_sizes()

    # margin d_c = |means_c / ||means_c|| |^2 (~1.0), matching the reference's
    # f32 normalize-then-dot on the label diagonal
    means = (centers / counts[:, None]).astype(np.float32)
    nrm = np.sqrt((means.astype(np.float32) ** 2).sum(axis=1, keepdims=True))
    mn = (means / nrm).astype(np.float32)
    dsq = (mn.astype(np.float64) ** 2).sum(axis=1)       # [C]
    d = dsq[labels_i]                                    # [B]

    S_tot = np.zeros(B, dtype=np.float64)
    t = np.empty(B, dtype=np.float64)
    for p in range(NCORES):
        # outS[q, b] is row b*128 + q
        S_p = res.results[p]["outS"].astype(np.float64).T.reshape(B)
        S_tot += S_p - float(CSH - sizes[p])             # remove zero-pad exp(0)=1
        # outT[q, j] is row p*BSH + j*128 + q
        T_p = res.results[p]["outT"].astype(np.float64)  # [128, JT]
        t[p * BSH:(p + 1) * BSH] = T_p.T.reshape(BSH)

    z = S_tot - np.exp(t) + np.exp(t + d)
    nll = np.log(z) - (t + d)
    loss = np.float32(nll.mean())
    out = np.array(loss, dtype=np.float32)
    if _ret_res:
        return out, res
    return out


# revision 16
# speedup vs baseline: 2.1053x; 1.3974x over previous
"""Trainium2 Bass kernel for a margin-softmax cross-entropy loss.

Reference computation (B=4096, D=512, C=10575):
    original = feats @ w                         # [B, C]
    means    = centers / counts[:, None]
    mn       = means / ||means||                 # unit rows
    dists    = mn[labels] @ mn.T                 # [B, C]
    logits   = original + onehot(labels) * dists # only label column changes
    loss     = mean(CE(logits, labels))

Key identities used:
  * Only the label column of `dists` survives the onehot mask, and
    dists[i, labels[i]] = mn[labels[i]] . mn[labels[i]] = |mn|^2 ~ 1.0
    (computed per-class on host from centers/counts; the BxC `dists`
    GEMM never needs to run).
  * logits are bounded (|logit| < ~3), so sum(exp(logits)) needs no
    max-subtraction; CE = log(sum_j exp(l_j)) - l_label.
  * Cross-entropy needs only two per-row scalars from the big [B, C]
    logits matrix: S_i = sum_j exp(l_ij) and t_i = l_i,label(i).

Device work per core (classes sharded 8 ways, ~1322 classes/core padded
to 1536): one [4096 x 512] @ [512 x 1536] bf16 GEMM with exp+row-sum
fused on the Scalar engine (activation accum_out), plus a tiny
"diagonal GEMM" for the label logits: with wlab = w[:, labels] gathered
on host for this core's 512-row batch slice, diag(featsT.T @ wlab) is
exactly t_i; the diagonal is pulled out with a constant identity mask
(one multiply + one reduce on 128x128 tiles).

Host combines the 8 partial sums (trivial [4096]-vector math) and
applies the margin correction:
    S' = S - exp(t) + exp(t + d);  nll = log(S') - (t + d).
Zero-padded w columns produce logits == 0 exactly, contributing
exp(0) = 1 each; the host subtracts the pad count from S.
"""

from contextlib import ExitStack

import ml_dtypes
import numpy as np

import concourse.bass as bass
import concourse.tile as tile
from concourse import bacc, mybir
from concourse.bass_utils import run_bass_kernel_spmd

B = 4096
D = 512
C = 10575
NCORES = 8
CS_BASE = 1322        # real classes on cores 0..6; core 7 gets 1321
CSH = 1328            # padded per-core class count
CW = (512, 512, 304)  # class-tile widths (PSUM bank-aligned starts)
CO = (0, 512, 1024)   # class-tile offsets
KT = D // 128         # 4 contraction tiles (bf16 diag GEMM)
KP = 2                # fp8 DoubleRow k-pairs (256 contraction each)
BT = B // 128         # 32 batch tiles
BSH = B // NCORES     # 512 rows of label logits per core
JT = BSH // 128       # 4 diagonal sub-tiles
FCH = 2               # fT DMA chunks per k-tile
WSCALE = 64.0         # fp8 pre-scale for w (subnormal-range fix), undone in exp

BF16 = mybir.dt.bfloat16
FP8 = mybir.dt.float8e4
F32 = mybir.dt.float32

_CACHE = {}


def _build_nc():
    nc = bacc.Bacc("TRN2", debug=False, target_bir_lowering=False)

    fT = nc.dram_tensor("fT", [KP, 128, 2, B], FP8, kind="ExternalInput").ap()
    wS = nc.dram_tensor("wS", [KP, 128, 2, CSH], FP8, kind="ExternalInput").ap()
    # wL and fS concatenated: [KT, 128, 0:BSH]=wL, [KT, 128, BSH:2*BSH]=fS
    wLfS = nc.dram_tensor("wLfS", [KT, 128, 2 * BSH], BF16, kind="ExternalInput").ap()
    ident = nc.dram_tensor("ident", [128, 128], F32, kind="ExternalInput").ap()
    outS = nc.dram_tensor("outS", [128, BT], F32, kind="ExternalOutput").ap()
    outT = nc.dram_tensor("outT", [128, JT], F32, kind="ExternalOutput").ap()

    with tile.TileContext(nc) as tc, ExitStack() as ctx:
        consts = ctx.enter_context(tc.tile_pool(name="consts", bufs=1))
        psums = ctx.enter_context(tc.tile_pool(name="psums", bufs=2, space="PSUM"))
        psumd = ctx.enter_context(tc.tile_pool(name="psumd", bufs=2, space="PSUM"))
        work = ctx.enter_context(tc.tile_pool(name="work", bufs=3))
        outs = ctx.enter_context(tc.tile_pool(name="outs", bufs=1))

        # scalar-engine HWDGE ring: weights + diag inputs (small, needed first)
        wS_sb = []
        for k in range(KP):
            t = consts.tile([128, 2, CSH], FP8, tag=f"wS{k}")
            nc.scalar.dma_start(out=t[:], in_=wS[k])
            wS_sb.append(t)
        wLfS_sb = []
        for k in range(KT):
            t = consts.tile([128, 2 * BSH], BF16, tag=f"wLfS{k}")
            nc.scalar.dma_start(out=t[:], in_=wLfS[k])
            wLfS_sb.append(t)
        id_sb = consts.tile([128, 128], F32, tag="ident")
        nc.scalar.dma_start(out=id_sb[:], in_=ident[:])
        wL_sb = [t[:, 0:BSH] for t in wLfS_sb]
        fS_sb = [t[:, BSH:2 * BSH] for t in wLfS_sb]
        # sync-engine HWDGE ring: feats transposed, two halves per k-pair so
        # the first b-tiles unblock at ~half the load time
        fT_sb = []
        bch = B // FCH
        for k in range(KP):
            t = consts.tile([128, 2, B], FP8, tag=f"fT{k}")
            fT_sb.append(t)
        for j in range(FCH):
            for k in range(KP):
                nc.sync.dma_start(
                    out=fT_sb[k][:, :, j * bch:(j + 1) * bch],
                    in_=fT[k][:, :, j * bch:(j + 1) * bch],
                )

        # label logits: diag(fS.T @ wL) on 128x128 sub-tiles
        tt = outs.tile([128, JT], F32, tag="tt")
        for j in range(JT):
            pd = psumd.tile([128, 128], F32, tag="pd")
            for k in range(KT):
                nc.tensor.matmul(
                    out=pd[:],
                    lhsT=fS_sb[k][:, j * 128:(j + 1) * 128],
                    rhs=wL_sb[k][:, j * 128:(j + 1) * 128],
                    start=(k == 0),
                    stop=(k == KT - 1),
                )
            scr = work.tile([128, 128], F32, tag="scr")
            nc.vector.tensor_mul(out=scr[:], in0=id_sb[:], in1=pd[:])
            nc.vector.tensor_reduce(
                out=tt[:, j:j + 1], in_=scr[:],
                axis=mybir.AxisListType.X, op=mybir.AluOpType.add,
            )
        nc.sync.dma_start(out=outT[:], in_=tt[:])

        # main GEMM + exp row-sums (one wide ACT per b-tile strip);
        # per-b sums land in one [128, BT] tile, stored with a single DMA
        st = outs.tile([128, BT], F32, tag="st")
        for b in range(BT):
            ps = psums.tile([128, CSH], F32, tag="ps")
            for k in range(KP):
                for c in range(len(CW)):
                    nc.tensor.matmul(
                        out=ps[:, CO[c]:CO[c] + CW[c]],
                        lhsT=fT_sb[k][:, :, b * 128:(b + 1) * 128],
                        rhs=wS_sb[k][:, :, CO[c]:CO[c] + CW[c]],
                        start=(k == 0),
                        stop=(k == KP - 1),
                        perf_mode=mybir.MatmulPerfMode.DoubleRow,
                    )
            e = work.tile([128, CSH], BF16, tag="e")
            nc.scalar.activation(
                out=e[:],
                in_=ps[:],
                func=mybir.ActivationFunctionType.Exp,
                scale=float(1.0 / WSCALE),
                accum_out=st[:, b:b + 1],
            )
        nc.sync.dma_start(out=outS[:], in_=st[:])

    nc.compile()
    return nc


def _core_sizes():
    sizes = [CS_BASE] * (NCORES - 1) + [C - CS_BASE * (NCORES - 1)]
    starts = np.concatenate([[0], np.cumsum(sizes)[:-1]]).astype(np.int64)
    return np.array(sizes, dtype=np.int64), starts


def _prepare_inputs(feats, labels, w):
    sizes, starts = _core_sizes()
    # fp8 DoubleRow layout: element [kp, p, i, b] = feats[b, kp*256 + i*128 + p]
    fT_host = np.ascontiguousarray(
        feats.reshape(B, KP, 2, 128).transpose(1, 3, 2, 0)
    ).astype(ml_dtypes.float8_e4m3)
    ident = np.eye(128, dtype=np.float32)

    in_maps = []
    for p in range(NCORES):
        c0, sz = int(starts[p]), int(sizes[p])
        wp = np.zeros((D, CSH), dtype=np.float32)
        wp[:, :sz] = w[:, c0:c0 + sz] * WSCALE
        wS_host = np.ascontiguousarray(
            wp.reshape(KP, 2, 128, CSH).transpose(0, 2, 1, 3)
        ).astype(ml_dtypes.float8_e4m3)

        rows = slice(p * BSH, (p + 1) * BSH)
        wlab = w[:, labels[rows]].reshape(KT, 128, BSH)           # [D, BSH]
        fsel = feats[rows].reshape(BSH, KT, 128).transpose(1, 2, 0)
        wLfS_host = np.concatenate([wlab, fsel], axis=2)
        wLfS_host = np.ascontiguousarray(wLfS_host).astype(ml_dtypes.bfloat16)

        in_maps.append({
            "fT": fT_host,
            "wS": wS_host,
            "wLfS": wLfS_host,
            "ident": ident,
        })
    return in_maps


def _run(in_maps, trace=False):
    if "nc" not in _CACHE:
        _CACHE["nc"] = _build_nc()
    nc = _CACHE["nc"]
    return run_bass_kernel_spmd(
        nc, in_maps, core_ids=list(range(NCORES)), trace=trace
    )


def kernel(feats, labels, centers, counts, w, _trace=False, _ret_res=False):
    feats = np.asarray(feats, dtype=np.float32)
    labels_i = np.asarray(labels).astype(np.int64)
    centers = np.asarray(centers, dtype=np.float32)
    counts = np.asarray(counts, dtype=np.float32)
    w = np.asarray(w, dtype=np.float32)

    in_maps = _prepare_inputs(feats, labels_i, w)
    res = _run(in_maps, trace=_trace)

    sizes, starts = _core_sizes()

    # margin d_c = |means_c / ||means_c|| |^2 (~1.0), matching the reference's
    # f32 normalize-then-dot on the label diagonal
    means = (centers / counts[:, None]).astype(np.float32)
    nrm = np.sqrt((means.astype(np.float32) ** 2).sum(axis=1, keepdims=True))
    mn = (means / nrm).astype(np.float32)
    dsq = (mn.astype(np.float64) ** 2).sum(axis=1)       # [C]
    d = dsq[labels_i]                                    # [B]

    S_tot = np.zeros(B, dtype=np.float64)
    t = np.empty(B, dtype=np.float64)
    for p in range(NCORES):
        # outS[q, b] is row b*128 + q
        S_p = res.results[p]["outS"].astype(np.float64).T.reshape(B)
        S_tot += S_p - float(CSH - sizes[p])             # remove zero-pad exp(0)=1
        # outT[q, j] is row p*BSH + j*128 + q
        T_p = res.results[p]["outT"].astype(np.float64)  # [128, JT]
        t[p * BSH:(p + 1) * BSH] = T_p.T.reshape(BSH)

    z = S_tot - np.exp(t) + np.exp(t + d)
    nll = np.log(z) - (t + d)
    loss = np.float32(nll.mean())
    out = np.array(loss, dtype=np.float32)
    if _ret_res:
        return out, res
    return out
